# revision 33
# baseline (speedup 1.0000x reference)
"""Trainium2 Bass kernel for nn_AttentionBlock (B=4, T=2048, C=1024, H=16, D=64).

Sharding over 8 NeuronCores: core c -> (batch b = c//2, head-half hg = c%2).
Each core runs LN1 + QKV (its 8 heads, all 2048 tokens of its batch) + causal
attention, then a paired AllGather swaps head-shards -> token-shards, and each
core runs proj + residual + LN2 + FFN + residual for its 1024 tokens.

Everything on-chip is stored feature-major ("transposed": features on SBUF
partitions), so no transposes are needed anywhere:
  - LN stats (sums over features = partitions) via ones-vector matmuls on PE
  - q^T,k^T head-dim-major; V token-major  (both directly from projections)
  - scores computed as S^T = K @ Q^T with 2 heads row-packed (contraction 64)
  - exp on ScalarE (scores bounded ~|2|, so no max-subtraction needed)
  - P@V with V stationary, 2 heads column-packed; softmax denominators via
    ones-column matmuls accumulated in PSUM
All matmul inputs bf16 (fp32 PSUM accumulation); the residual spine stays fp32.
"""

import os
import threading

import numpy as np
import ml_dtypes

import concourse.bacc as bacc
import concourse.bass as bass
import concourse.mybir as mybir
import concourse.tile as tile
from concourse.bass import ds, ts

F32 = mybir.dt.float32
BF16 = mybir.dt.bfloat16
AF = mybir.ActivationFunctionType
OP = mybir.AluOpType

B, T, C, H, D = 4, 2048, 1024, 16, 64
FF = 4 * C
NCORES = 8
P = 128
CK = C // P            # 8 feature chunks
MYT = T // 2           # tokens per core after the exchange
LN_EPS = 1e-6
NEG = -1.0e30
DEBUG_TAPS = bool(int(__import__("os").environ.get("KERNEL_DEBUG_TAPS", "0")))
SIM_SINGLE = bool(int(__import__("os").environ.get("KERNEL_SIM_SINGLE", "0")))

_lock = threading.Lock()
_cache: dict = {}


# ----------------------------------------------------------------------------
# kernel builder
# ----------------------------------------------------------------------------

def _build_nc():
    nc = bacc.Bacc(
        "TRN2",
        target_bir_lowering=False,
        debug=False,
        num_devices=1 if SIM_SINGLE else NCORES,
    )

    dram = {}

    def din(name, shape, dt):
        dram[name] = nc.dram_tensor(name, shape, dt, kind="ExternalInput")
        return dram[name]

    xT_d = din("xT", [C, T], F32)
    xTmine_d = din("xTmine", [C, MYT], F32)
    wq_d = din("wq", [C, 512], BF16)
    wk_d = din("wk", [C, 512], BF16)
    wv_d = din("wv", [C, 512], BF16)
    wproj_d = din("wproj", [C, C], BF16)
    wff1_d = din("wff1", [C, FF], BF16)
    wff2_d = din("wff2", [FF, C], BF16)
    bqkv_d = din("bqkv", [P, 12], F32)
    bproj_d = din("bproj", [P, 8], F32)
    bff1_d = din("bff1", [P, 32], F32)
    bff2_d = din("bff2", [P, 8], F32)
    trimask_d = din("trimask", [P, P], F32)
    onesb_d = din("onesb", [P, 1], BF16)
    yT_d = nc.dram_tensor("yT", [C, MYT], F32, kind="ExternalOutput")
    dbg = {}
    if DEBUG_TAPS:
        dbg["xn"] = nc.dram_tensor("dbg_xn", [P, CK * T], BF16, kind="ExternalOutput")
        dbg["qT"] = nc.dram_tensor("dbg_qT", [P, 4 * T], BF16, kind="ExternalOutput")
        dbg["kT"] = nc.dram_tensor("dbg_kT", [P, 4 * T], BF16, kind="ExternalOutput")
        dbg["v"] = nc.dram_tensor("dbg_v", [P, 16 * 512], BF16, kind="ExternalOutput")
        dbg["attn"] = nc.dram_tensor("dbg_attn", [P, 4 * T], BF16, kind="ExternalOutput")
        dbg["x1"] = nc.dram_tensor("dbg_x1", [P, CK * MYT], F32, kind="ExternalOutput")
        dbg["s2"] = nc.dram_tensor("dbg_s2", [P, 4 * 1024], F32, kind="ExternalOutput")
        dbg["p2"] = nc.dram_tensor("dbg_p2", [P, 4 * 1024], BF16, kind="ExternalOutput")
        dbg["rdb"] = nc.dram_tensor("dbg_rdb", [P, 512], F32, kind="ExternalOutput")
        dbg["dd"] = nc.dram_tensor("dbg_dd", [P, 2048], F32, kind="ExternalOutput")

    with tile.TileContext(nc) as tc:
        _emit(nc, tc, dram, yT_d, dbg)
    nc.compile()
    return nc


def _emit(nc, tc, dram, yT_d, dbg):
    sync, vec, act, gps, pe = nc.sync, nc.vector, nc.scalar, nc.gpsimd, nc.tensor

    pid = nc.sync.partition_id()
    off_mine = (pid % 2) * MYT          # my token half, in columns
    off_send = MYT - (pid % 2) * MYT    # partner's token half
    slot_partner = 1 - (pid % 2)        # partner's rank slot in the AllGather

    def layernorm_stats(xsrc, ntok, statp, smp, prefix, on_ready=None):
        """xsrc: callable (ck, tcx) -> bf16 [P, 512] tile AP for that chunk.
        Returns per-512-token-chunk (mean_b, rstd_b) f32 [P, 512] broadcast tiles,
        emitted tcx-granular so downstream consumers of early chunks can start
        while later chunks' stats are still being accumulated."""
        ntc = ntok // 512
        out = []
        for tcx in range(ntc):
            stat = statp.tile([64, 512], F32, tag="stat", name=f"{prefix}st{tcx}")
            for ck in range(CK):
                xbt = xsrc(ck, tcx)
                xsq = smp.tile([P, 512], BF16, tag="xsq", name=f"{prefix}xsq{tcx}_{ck}")
                act.square(xsq[:], xbt[:])
                pe.matmul(stat[0:1, :], onesb[:], xbt[:],
                          start=(ck == 0), stop=False)
                pe.matmul(stat[32:33, :], onesb[:], xsq[:],
                          start=(ck == 0), stop=(ck == CK - 1),
                          skip_group_check=True)
            mean = smp.tile([1, 512], F32, tag="sm", name=f"{prefix}mean{tcx}")
            act.mul(mean[:], stat[0:1, :], 1.0 / C)
            msq = smp.tile([1, 512], F32, tag="sm", name=f"{prefix}msq{tcx}")
            act.mul(msq[:], stat[32:33, :], 1.0 / C)
            var = smp.tile([1, 512], F32, tag="sm", name=f"{prefix}var{tcx}")
            act.square(var[:], mean[:])
            vec.tensor_tensor(var[:], msq[:], var[:], OP.subtract)
            std = smp.tile([1, 512], F32, tag="sm", name=f"{prefix}std{tcx}")
            act.activation(std[:], var[:], AF.Sqrt, bias=_eps[0:1, :])
            rstd = smp.tile([1, 512], F32, tag="sm", name=f"{prefix}rstd{tcx}")
            vec.reciprocal_approx_fast(rstd[:], std[:])
            mb = smp.tile([P, 512], F32, tag=f"mb{tcx}", name=f"{prefix}mb{tcx}", bufs=1)
            rb = smp.tile([P, 512], F32, tag=f"rb{tcx}", name=f"{prefix}rb{tcx}", bufs=1)
            gps.partition_broadcast(mb[:], mean[:])
            gps.partition_broadcast(rb[:], rstd[:])
            out.append((mb, rb))
            if on_ready is not None:
                on_ready(tcx, mb, rb)
        return out

    with (
        tc.tile_pool(name="const", bufs=1) as constp,
        tc.tile_pool(name="mid", bufs=1) as midp,
        tc.tile_pool(name="dram", bufs=1, space="DRAM") as dramp,
    ):
        trimask = constp.tile([P, P], F32)
        sync.dma_start(trimask[:], dram["trimask"].ap())
        onesb = constp.tile([P, 1], BF16)
        sync.dma_start(onesb[:], dram["onesb"].ap())
        bqkv = constp.tile([P, 12], F32)
        sync.dma_start(bqkv[:], dram["bqkv"].ap())
        bproj = constp.tile([P, 8], F32)
        sync.dma_start(bproj[:], dram["bproj"].ap())
        bff1 = constp.tile([P, 32], F32)
        sync.dma_start(bff1[:], dram["bff1"].ap())
        bff2 = constp.tile([P, 8], F32)
        sync.dma_start(bff2[:], dram["bff2"].ap())
        epst = constp.tile([P, 1], F32)
        vec.memset(epst[:], LN_EPS)
        _eps = epst

        def _ones(_ck):
            return onesb[:]

        # tensors that must outlive the attention scope
        amine3 = midp.tile([P, 4, MYT], BF16, tag="amine")
        apart3 = midp.tile([P, 4, MYT], BF16, tag="apart")
        xn23 = midp.tile([P, CK, MYT], BF16, tag="xn2")

        with tc.tile_pool(name="attnspan", bufs=1) as attnsp:
            attn3 = attnsp.tile([P, 4, T], BF16)

            with tc.tile_pool(name="qkvxn", bufs=1) as qxp:
                qT3 = qxp.tile([P, 4, T], BF16, tag="qT3")
                kT3 = qxp.tile([P, 4, T], BF16, tag="kT3")
                v3 = qxp.tile([P, 16, 512], BF16, tag="v3")

                with (
                    tc.tile_pool(name="span_xn", bufs=1) as xnp,
                    tc.tile_pool(name="ln1_rot", bufs=2) as lp,
                    tc.tile_pool(name="ln1_small", bufs=5) as smp,
                    tc.tile_pool(name="ln1_stats", bufs=2, space="PSUM") as statp,
                    tc.tile_pool(name="wqkv", bufs=1) as wp,
                    tc.tile_pool(name="qkv_ps", bufs=4, space="PSUM") as qp,
                ):
                    xn3 = xnp.tile([P, CK, T], BF16, tag="xn3")
                    wq = wp.tile([P, CK, 512], BF16, tag="wq")
                    sync.dma_start(wq[:], dram["wq"].ap().rearrange("(ck p) n -> p ck n", p=P))
                    wk = wp.tile([P, CK, 512], BF16, tag="wk")
                    sync.dma_start(wk[:], dram["wk"].ap().rearrange("(ck p) n -> p ck n", p=P))
                    wv = wp.tile([P, CK, 512], BF16, tag="wv")
                    sync.dma_start(wv[:], dram["wv"].ap().rearrange("(ck p) n -> p ck n", p=P))

                    def xsrc1(ck, tcx):
                        xt = lp.tile([P, 512], F32, tag="xT", name=f"xt{tcx}_{ck}")
                        sync.dma_start(xt[:], dram["xT"].ap()[ts(ck, P), ts(tcx, 512)])
                        xbt = lp.tile([P, 512], BF16, tag="xb", name=f"xb{tcx}_{ck}")
                        vec.tensor_copy(xbt[:], xt[:])
                        return xbt

                    def ln1_ready(tcx, mb, rb):
                        # normalize this 512-token chunk, then its QKV slice
                        for ck in range(CK):
                            xt = lp.tile([P, 512], F32, tag="xT", name=f"xtn{tcx}_{ck}")
                            sync.dma_start(xt[:], dram["xT"].ap()[ts(ck, P), ts(tcx, 512)])
                            tmp = lp.tile([P, 512], BF16, tag="lntmp", name=f"lt{ck}_{tcx}")
                            vec.tensor_tensor(tmp[:], xt[:], mb[:], OP.subtract)
                            vec.tensor_tensor(xn3[:, ck, ts(tcx, 512)], tmp[:], rb[:], OP.mult)
                        for ti, (wt, out3) in enumerate(((wq, qT3), (wk, kT3))):
                            for m in range(4):
                                ps = qp.tile([P, 512], F32, tag="qkps",
                                             name=f"qk{ti}_{m}_{tcx}")
                                for ck in range(CK):
                                    pe.matmul(ps[:], wt[:, ck, ts(m, P)],
                                              xn3[:, ck, ts(tcx, 512)],
                                              start=(ck == 0), stop=(ck == CK - 1))
                                vec.tensor_scalar_add(out3[:, m, ts(tcx, 512)], ps[:],
                                                      bqkv[:, ti * 4 + m : ti * 4 + m + 1])
                        for kt in range(4 * tcx, 4 * tcx + 4):
                            ps = qp.tile([P, 512], F32, tag="qkps", name=f"v{kt}")
                            for ck in range(CK):
                                pe.matmul(ps[:], xn3[:, ck, ts(kt, P)], wv[:, ck, :],
                                          start=(ck == 0), stop=(ck == CK - 1))
                            vec.tensor_copy(v3[:, kt, :], ps[:])

                    layernorm_stats(xsrc1, T, statp, smp, "l1", on_ready=ln1_ready)

                # ----------------------------------------------------------
                # Phase 3: causal attention per (head-pair hp, q-tile qt)                # ----------------------------------------------------------
                # Phase 3: causal attention per (head-pair hp, q-tile qt)
                # ----------------------------------------------------------
                cins = [dramp.tile([P, MYT], BF16, name=f"cin{h_}") for h_ in range(4)]
                couts = [dramp.tile([2, P, MYT], BF16, name=f"cout{h_}") for h_ in range(4)]
                w1cm = tc.tile_pool(name="w1", bufs=1, side="right")
                w1p = w1cm.__enter__()
                w1 = w1p.tile([P, CK, FF], BF16)
                sync.dma_start(w1[:], dram["wff1"].ap().rearrange("(ck p) n -> p ck n", p=P))
                with (
                    tc.tile_pool(name="p2", bufs=3) as p2p,
                    tc.tile_pool(name="rd", bufs=2) as rdp,
                    tc.tile_pool(name="ps_s", bufs=2, space="PSUM") as psS,
                    tc.tile_pool(name="ps_attn", bufs=2, space="PSUM") as psA,
                    tc.tile_pool(name="ps_d", bufs=1, space="PSUM") as psD,
                ):
                    for hp in range(4):
                        for qt in range(4):
                            t0 = qt * 512
                            kdiag = 4 * qt
                            chunks = ([kdiag] + list(range(kdiag))
                                      + [kdiag + 1, kdiag + 2, kdiag + 3])
                            attn2 = psA.tile([P, 512], F32, tag="attn2", bufs=1,
                                              name=f"at{hp}_{qt}")
                            attn2b = psA.tile([P, 512], F32, tag="attn2b", bufs=1,
                                              name=f"atb{hp}_{qt}")
                            d2 = psD.tile([64, 512], F32, tag="d2", bufs=2,
                                          name=f"d{hp}_{qt}")
                            s_tiles = {}

                            def qk(k):
                                off = 0 if k <= kdiag else P * (k - kdiag)
                                s2 = psS.tile([P, 1024], F32, tag="s2",
                                              name=f"s{hp}_{qt}_{k}")
                                pe.matmul(s2[:, off:512],
                                          kT3[0:64, hp, ts(k, P)],
                                          qT3[0:64, hp, t0 + off : t0 + 512],
                                          start=True, stop=True, tile_position=(0, 0))
                                pe.matmul(s2[:, 512 + off : 1024],
                                          kT3[64:128, hp, ts(k, P)],
                                          qT3[64:128, hp, t0 + off : t0 + 512],
                                          start=True, stop=True, tile_position=(64, 0))
                                s_tiles[k] = s2

                            qk(chunks[0])
                            if len(chunks) > 1:
                                qk(chunks[1])
                            for i, k in enumerate(chunks):
                                off = 0 if k <= kdiag else P * (k - kdiag)
                                first = i == 0
                                last = i == len(chunks) - 1
                                s2 = s_tiles.pop(k)
                                if k >= kdiag:  # diagonal chunk: triangular mask
                                    vec.tensor_tensor(s2[:, off : off + P],
                                                      s2[:, off : off + P],
                                                      trimask[:], OP.add)
                                    vec.tensor_tensor(s2[:, 512 + off : 512 + off + P],
                                                      s2[:, 512 + off : 512 + off + P],
                                                      trimask[:], OP.add)
                                p2 = p2p.tile([P, 1024], BF16, tag="p2",
                                              name=f"p{hp}_{qt}_{k}")
                                if dbg and hp == 0 and qt == 0:
                                    s2c = p2p.tile([P, 1024], F32, tag="s2c",
                                                   name=f"s2c{k}")
                                    vec.tensor_copy(s2c[:], s2[:])
                                    sync.dma_start(
                                        dbg["s2"].ap().rearrange("p (k n) -> p k n", k=4)[:, k, :],
                                        s2c[:])
                                if off == 0:
                                    act.activation(p2[:, 0:1024], s2[:, 0:1024], AF.Exp)
                                else:
                                    act.activation(p2[:, off:512], s2[:, off:512], AF.Exp)
                                    act.activation(p2[:, 512 + off : 1024],
                                                   s2[:, 512 + off : 1024], AF.Exp)
                                if dbg and hp == 0 and qt == 0:
                                    sync.dma_start(
                                        dbg["p2"].ap().rearrange("p (k n) -> p k n", k=4)[:, k, :],
                                        p2[:])
                                if i + 2 < len(chunks):
                                    qk(chunks[i + 2])
                                vw = v3[:, k, hp * P : hp * P + P]
                                pe.matmul(attn2[:, off:512], vw, p2[:, off:512],
                                          start=first, stop=last)
                                pe.matmul(attn2b[:, off:512], vw,
                                          p2[:, 512 + off : 1024],
                                          start=first, stop=last)
                                pe.matmul(d2[0:1, off:512], onesb[:], p2[:, off:512],
                                          start=first, stop=last,
                                          tile_position=(0, 0))
                                pe.matmul(d2[32:33, off:512], onesb[:],
                                          p2[:, 512 + off : 1024],
                                          start=first, stop=last,
                                          tile_position=(0, 32), skip_group_check=True)

                            dsA = rdp.tile([1, 512], F32, tag="dsA", name=f"ca{hp}_{qt}")
                            dsB = rdp.tile([1, 512], F32, tag="dsB", name=f"cb{hp}_{qt}")
                            vec.tensor_copy(dsA[0:1, :], d2[0:1, :])
                            vec.tensor_copy(dsB[0:1, :], d2[32:33, :])
                            rdA = rdp.tile([1, 512], F32, tag="rdA", name=f"ra{hp}_{qt}")
                            rdB = rdp.tile([1, 512], F32, tag="rdB", name=f"rB{hp}_{qt}")
                            vec.reciprocal_approx_fast(rdA[0:1, :], dsA[0:1, :])
                            vec.reciprocal_approx_fast(rdB[0:1, :], dsB[0:1, :])
                            rdbA = rdp.tile([P, 512], F32, tag="rdbA", name=f"rba{hp}_{qt}")
                            rdbB = rdp.tile([P, 512], F32, tag="rdbB", name=f"rbb{hp}_{qt}")
                            gps.partition_broadcast(rdbA[:, :], rdA[0:1, :])
                            gps.partition_broadcast(rdbB[:, :], rdB[0:1, :])
                            if dbg and hp == 0 and qt == 0:
                                sync.dma_start(dbg["rdb"].ap()[0:64, :], rdbA[0:64, :])
                                sync.dma_start(dbg["rdb"].ap()[64:128, :], rdbB[64:128, :])
                                ddc = rdp.tile([1, 2048], F32, tag="ddc", name="ddc0")
                                vec.tensor_copy(ddc[0:1, 0:512], dsA[0:1, :])
                                vec.tensor_copy(ddc[0:1, 512:1024], dsB[0:1, :])
                                vec.tensor_copy(ddc[0:1, 1024:1536], rdA[0:1, :])
                                vec.tensor_copy(ddc[0:1, 1536:2048], rdB[0:1, :])
                                sync.dma_start(dbg["dd"].ap()[0:1, :], ddc[0:1, :])
                            vec.tensor_tensor(attn3[0:64, hp, ts(qt, 512)],
                                              attn2[0:64, :], rdbA[0:64, :], OP.mult)
                            vec.tensor_tensor(attn3[64:128, hp, ts(qt, 512)],
                                              attn2b[64:128, :], rdbB[64:128, :], OP.mult)

                        # per-hp exchange as soon as this head-pair finishes
                        sync.dma_start(cins[hp][:, :], attn3[:, hp, ds(off_send, MYT)])
                        sync.dma_start(amine3[:, hp, :], attn3[:, hp, ds(off_mine, MYT)])
                        if SIM_SINGLE:
                            sync.dma_start(couts[hp][0], cins[hp][:])
                            sync.dma_start(couts[hp][1], cins[hp][:])
                        else:
                            gps.collective_compute(
                                "AllGather",
                                OP.bypass,
                                ins=[cins[hp][:].opt()],
                                outs=[couts[hp][:].opt()],
                                replica_groups=[[0, 1], [2, 3], [4, 5], [6, 7]],
                            )

            if dbg:
                sync.dma_start(dbg["attn"].ap().rearrange("p (m t) -> p m t", m=4), attn3[:])

            # --------------------------------------------------------------
            # Phase 4 tail: read back per-hp AllGather results
            # --------------------------------------------------------------
            for hp in range(4):
                sync.dma_start(apart3[:, hp, :],
                               couts[hp][ds(slot_partner, 1), :, :])

        # ------------------------------------------------------------------
        # Phase 5-7: proj + residual, LN2, FFN  (my MYT tokens)
        # ------------------------------------------------------------------
        with tc.tile_pool(name="x1span", bufs=1) as x1p:
            x1T3 = x1p.tile([P, CK, MYT], F32)

            with (
                tc.tile_pool(name="wproj", bufs=1) as wpp,
                tc.tile_pool(name="xmine", bufs=1) as xmp,
                tc.tile_pool(name="proj_ps", bufs=4, space="PSUM") as pp,
            ):
                wproj = wpp.tile([P, CK, C], BF16)
                for r_ in range(CK):
                    sync.dma_start(wproj[:, r_, :], dram["wproj"].ap()[ts(r_, P), :])
                xmine = xmp.tile([P, CK, MYT], F32)
                for r_ in range(CK):
                    sync.dma_start(xmine[:, r_, :], dram["xTmine"].ap()[ts(r_, P), :])
                for tc2 in range(2):
                    for m in range(CK):
                        ps = pp.tile([P, 512], F32, tag="projps", name=f"pj{m}_{tc2}")
                        for r in range(4):
                            pe.matmul(ps[:], wproj[:, r, ts(m, P)],
                                      amine3[:, r, ts(tc2, 512)],
                                      start=(r == 0), stop=False)
                        for r in range(4):
                            pe.matmul(ps[:], wproj[:, r + 4, ts(m, P)],
                                      apart3[:, r, ts(tc2, 512)],
                                      start=False, stop=(r == 3))
                        vec.scalar_tensor_tensor(
                            x1T3[:, m, ts(tc2, 512)], ps[:], bproj[:, m : m + 1],
                            xmine[:, m, ts(tc2, 512)], op0=OP.add, op1=OP.add)

            if dbg:
                sync.dma_start(dbg["x1"].ap().rearrange("p (ck t) -> p ck t", ck=CK), x1T3[:])

            # LN2
            with (
                tc.tile_pool(name="ln2_rot", bufs=2) as lp2,
                tc.tile_pool(name="ln2_xb", bufs=1) as xb2p,
                tc.tile_pool(name="ln2_small", bufs=5) as smp2,
                tc.tile_pool(name="ln2_stats", bufs=2, space="PSUM") as statp2,
            ):
                xb23 = xb2p.tile([P, CK, MYT], BF16)

                def xsrc2(ck, tcx):
                    vec.tensor_copy(xb23[:, ck, ts(tcx, 512)], x1T3[:, ck, ts(tcx, 512)])
                    return xb23[:, ck, ts(tcx, 512)]

                def ln2_ready(tcx, mb, rb):
                    for ck in range(CK):
                        tmp = lp2.tile([P, 512], BF16, tag="lntmp2", name=f"l2t{ck}_{tcx}")
                        vec.tensor_tensor(tmp[:], x1T3[:, ck, ts(tcx, 512)], mb[:],
                                          OP.subtract)
                        vec.tensor_tensor(xn23[:, ck, ts(tcx, 512)], tmp[:], rb[:],
                                          OP.mult)
                layernorm_stats(xsrc2, MYT, statp2, smp2, "l2", on_ready=ln2_ready)

            # FFN
            with (
                tc.tile_pool(name="w2rot", bufs=3) as w2p,
                tc.tile_pool(name="h", bufs=1) as hp_,
                tc.tile_pool(name="yrot", bufs=3) as yp,
                tc.tile_pool(name="ffn_ps", bufs=2, space="PSUM") as fp,
                tc.tile_pool(name="ffn2_ps", bufs=1, space="PSUM") as fp2,
            ):
                for tc2 in range(2):
                    h3 = hp_.tile([P, 32, 512], BF16, tag="h3", name=f"h{tc2}")
                    for fm in range(32):
                        ps = fp.tile([P, 512], F32, tag="ffps", name=f"f{tc2}_{fm}")
                        for ck in range(CK):
                            pe.matmul(ps[:], w1[:, ck, ts(fm, P)],
                                      xn23[:, ck, ts(tc2, 512)],
                                      start=(ck == 0), stop=(ck == CK - 1))
                        act.activation(h3[:, fm, :], ps[:], AF.Relu,
                                       bias=bff1[:, fm : fm + 1])
                    for g in range(2):
                        ps_y = [fp2.tile([P, 512], F32, tag=f"yps{j}", bufs=1,
                                         name=f"y{tc2}_{g}_{j}") for j in range(4)]
                        for fk in range(32):
                            w2t = w2p.tile([P, C], BF16, tag="w2t", name=f"w2_{tc2}_{g}_{fk}")
                            sync.dma_start(w2t[:], dram["wff2"].ap()[ts(fk, P), :])
                            for j in range(4):
                                m2 = g * 4 + j
                                pe.matmul(ps_y[j][:], w2t[:, ts(m2, P)], h3[:, fk, :],
                                          start=(fk == 0), stop=(fk == 31))
                        for j in range(4):
                            m2 = g * 4 + j
                            yt = yp.tile([P, 512], F32, tag="yt", name=f"yo{tc2}_{g}_{j}")
                            vec.scalar_tensor_tensor(
                                yt[:], ps_y[j][:], bff2[:, m2 : m2 + 1],
                                x1T3[:, m2, ts(tc2, 512)], op0=OP.add, op1=OP.add)
                            sync.dma_start(yT_d.ap()[ts(m2, P), ts(tc2, 512)], yt[:])
            w1cm.__exit__(None, None, None)


# ----------------------------------------------------------------------------
# host-side input prep
# ----------------------------------------------------------------------------

def _bf16(a):
    return np.asarray(a, dtype=ml_dtypes.bfloat16)


def _prep_inputs(inputs):
    x = np.asarray(inputs["x"], np.float32)
    wq = np.asarray(inputs["wq"], np.float32)
    wk = np.asarray(inputs["wk"], np.float32)
    wv = np.asarray(inputs["wv"], np.float32)
    w_proj = np.asarray(inputs["w_proj"], np.float32)
    b_proj = np.asarray(inputs["b_proj"], np.float32)
    w_ff1 = np.asarray(inputs["w_ff1"], np.float32)
    b_ff1 = np.asarray(inputs["b_ff1"], np.float32)
    w_ff2 = np.asarray(inputs["w_ff2"], np.float32)
    b_ff2 = np.asarray(inputs["b_ff2"], np.float32)
    s1 = np.asarray(inputs["ln1_scale"], np.float32)
    b1 = np.asarray(inputs["ln1_bias"], np.float32)
    s2 = np.asarray(inputs["ln2_scale"], np.float32)
    b2 = np.asarray(inputs["ln2_bias"], np.float32)

    assert np.all(np.asarray(inputs["wv"]).shape == (H, C, D))
    scale = C ** -0.5

    # [H, C, D] -> [C, H*D], LN1 scale folded into rows; C^-0.5 folded into wq
    def flat(w, extra=1.0):
        return np.ascontiguousarray(w.transpose(1, 0, 2).reshape(C, H * D)) * extra

    wq_f = flat(wq, scale)        # [C, 1024]
    wk_f = flat(wk)
    wv_f = flat(wv)
    qb_full = b1 @ wq_f           # [1024] biases from ln1_bias (zero in practice)
    kb_full = b1 @ wk_f
    vb_full = b1 @ wv_f
    assert np.abs(vb_full).max() == 0.0, "nonzero ln1_bias @ wv not supported"
    wq_s = wq_f * s1[:, None]
    wk_s = wk_f * s1[:, None]
    wv_s = wv_f * s1[:, None]

    wff1_s = w_ff1 * s2[:, None]
    bff1_full = b2 @ w_ff1 + b_ff1    # [4096]

    trimask = np.where(np.arange(P)[:, None] <= np.arange(P)[None, :], 0.0, NEG
                       ).astype(np.float32)
    onesb = _bf16(np.ones((P, 1), np.float32))

    in_maps = []
    for c in range(NCORES):
        b, hg = c // 2, c % 2
        hsl = slice(hg * 512, hg * 512 + 512)
        perm = np.concatenate([np.arange(hg * 512, hg * 512 + 512),
                               np.arange((1 - hg) * 512, (1 - hg) * 512 + 512)])
        m = {
            "xT": np.ascontiguousarray(x[b].T),
            "xTmine": np.ascontiguousarray(x[b, hg * MYT:(hg + 1) * MYT].T),
            "wq": _bf16(wq_s[:, hsl]),
            "wk": _bf16(wk_s[:, hsl]),
            "wv": _bf16(wv_s[:, hsl]),
            "wproj": _bf16(w_proj[perm, :]),
            "wff1": _bf16(wff1_s),
            "wff2": _bf16(w_ff2),
            "bqkv": np.ascontiguousarray(np.concatenate(
                [qb_full[hsl].reshape(4, P).T,
                 kb_full[hsl].reshape(4, P).T,
                 np.zeros((P, 4), np.float32)], axis=1)),
            "bproj": np.ascontiguousarray(b_proj.reshape(8, P).T),
            "bff1": np.ascontiguousarray(bff1_full.reshape(32, P).T),
            "bff2": np.ascontiguousarray(b_ff2.reshape(8, P).T),
            "trimask": trimask,
            "onesb": onesb,
        }
        in_maps.append(m)
    return in_maps


# ----------------------------------------------------------------------------
# cached PJRT runner (one jit, reused across calls)
# ----------------------------------------------------------------------------

def _get_runner():
    with _lock:
        if "runner" in _cache:
            return _cache["runner"]

        import jax
        import jax.numpy as jnp
        from jax.sharding import Mesh, PartitionSpec
        from jax.experimental.shard_map import shard_map
        from concourse import bass2jax

        nc = _build_nc()
        bass2jax.install_neuronx_cc_hook()

        partition_name = nc.partition_id_tensor.name if nc.partition_id_tensor else None
        in_names, out_names, out_avals, zero_outs = [], [], [], []
        for alloc in nc.m.functions[0].allocations:
            if not isinstance(alloc, mybir.MemoryLocationSet):
                continue
            name = alloc.memorylocations[0].name
            if alloc.kind == "ExternalInput":
                if name != partition_name:
                    in_names.append(name)
            elif alloc.kind == "ExternalOutput":
                shape = tuple(alloc.tensor_shape)
                dtype = mybir.dt.np(alloc.dtype)
                out_names.append(name)
                out_avals.append(jax.core.ShapedArray(shape, dtype))
                zero_outs.append(np.zeros(shape, dtype))
        n_params = len(in_names)
        all_in_names = list(in_names) + list(out_names)
        if partition_name is not None:
            all_in_names.append(partition_name)
        donate = tuple(range(n_params, n_params + len(out_names)))

        def _body(*args):
            operands = list(args)
            if partition_name is not None:
                operands.append(bass2jax.partition_id_tensor())
            outs = bass2jax._bass_exec_p.bind(
                *operands,
                out_avals=tuple(out_avals),
                in_names=tuple(all_in_names),
                out_names=tuple(out_names),
                lowering_input_output_aliases=(),
                sim_require_finite=True,
                sim_require_nnan=True,
                nc=nc,
            )
            return tuple(outs)

        devices = jax.devices()[:NCORES]
        mesh = Mesh(np.asarray(devices), ("core",))
        in_specs = (PartitionSpec("core"),) * (n_params + len(out_names))
        out_specs = (PartitionSpec("core"),) * len(out_names)
        sharded = jax.jit(
            shard_map(_body, mesh=mesh, in_specs=in_specs, out_specs=out_specs,
                      check_rep=False),
            donate_argnums=donate, keep_unused=True,
        )

        from jax.sharding import NamedSharding

        def run(in_maps, timing_iters=0):
            per_core = [[np.asarray(m[k]) for k in in_names] for m in in_maps]
            concat_in = [np.concatenate([per_core[c][i] for c in range(NCORES)], axis=0)
                         for i in range(n_params)]
            shard = NamedSharding(mesh, PartitionSpec("core"))
            dev_in = [jax.device_put(a, shard) for a in concat_in]
            jax.block_until_ready(dev_in)

            def dev_zeros():
                zz = [jax.device_put(
                    np.zeros((NCORES * z.shape[0], *z.shape[1:]), z.dtype), shard)
                    for z in zero_outs]
                jax.block_until_ready(zz)
                return zz

            out = sharded(*dev_in, *dev_zeros())
            jax.block_until_ready(out)
            times = []
            if timing_iters:
                import time
                for _ in range(timing_iters):
                    zz = dev_zeros()
                    t0 = time.perf_counter()
                    o2 = sharded(*dev_in, *zz)
                    jax.block_until_ready(o2)
                    times.append(time.perf_counter() - t0)
                    out = o2
            res = [
                {name: np.asarray(out[i]).reshape(NCORES, *out_avals[i].shape)[c]
                 for i, name in enumerate(out_names)}
                for c in range(NCORES)
            ]
            return res, times

        _cache["runner"] = run
        return run


def kernel(**inputs) -> np.ndarray:
    run = _get_runner()
    in_maps = _prep_inputs(inputs)
    res, _ = run(in_maps)
    out = np.empty((B, T, C), np.float32)
    for c in range(NCORES):
        b, hg = c // 2, c % 2
        out[b, hg * MYT:(hg + 1) * MYT, :] = res[c]["yT"].T
    return out


def benchmark(inputs, iters=10):
    """Returns (output, list of per-iteration wall seconds)."""
    run = _get_runner()
    in_maps = _prep_inputs(inputs)
    res, times = run(in_maps, timing_iters=iters)
    out = np.empty((B, T, C), np.float32)
    for c in range(NCORES):
        b, hg = c // 2, c % 2
        out[b, hg * MYT:(hg + 1) * MYT, :] = res[c]["yT"].T
    return out, times


# revision 36
# speedup vs baseline: 1.8652x; 1.8652x over previous
"""Trainium2 Bass kernel for nn_AttentionBlock (B=4, T=2048, C=1024, H=16, D=64).

Sharding over 8 NeuronCores: core c -> (batch b = c//2, head-half hg = c%2).
Each core runs LN1 + QKV (its 8 heads, all 2048 tokens of its batch) + causal
attention, then a paired AllGather swaps head-shards -> token-shards, and each
core runs proj + residual + LN2 + FFN + residual for its 1024 tokens.

Everything on-chip is stored feature-major ("transposed": features on SBUF
partitions), so no transposes are needed anywhere:
  - LN stats (sums over features = partitions) via ones-vector matmuls on PE
  - q^T,k^T head-dim-major; V token-major  (both directly from projections)
  - scores computed as S^T = K @ Q^T with 2 heads row-packed (contraction 64)
  - exp on ScalarE (scores bounded ~|2|, so no max-subtraction needed)
  - P@V with V stationary, 2 heads column-packed; softmax denominators via
    ones-column matmuls accumulated in PSUM
All matmul inputs bf16 (fp32 PSUM accumulation); the residual spine stays fp32.
"""

import os
import threading

import numpy as np
import ml_dtypes

import concourse.bacc as bacc
import concourse.bass as bass
import concourse.mybir as mybir
import concourse.tile as tile
from concourse.bass import ds, ts

F32 = mybir.dt.float32
BF16 = mybir.dt.bfloat16
AF = mybir.ActivationFunctionType
OP = mybir.AluOpType

B, T, C, H, D = 4, 2048, 1024, 16, 64
FF = 4 * C
NCORES = 8
P = 128
CK = C // P            # 8 feature chunks
MYT = T // 2           # tokens per core after the exchange
LN_EPS = 1e-6
NEG = -1.0e30
DEBUG_TAPS = bool(int(__import__("os").environ.get("KERNEL_DEBUG_TAPS", "0")))
SIM_SINGLE = bool(int(__import__("os").environ.get("KERNEL_SIM_SINGLE", "0")))

_lock = threading.Lock()
_cache: dict = {}


# ----------------------------------------------------------------------------
# kernel builder
# ----------------------------------------------------------------------------

def _build_nc():
    nc = bacc.Bacc(
        "TRN2",
        target_bir_lowering=False,
        debug=False,
        num_devices=1 if SIM_SINGLE else NCORES,
    )

    dram = {}

    def din(name, shape, dt):
        dram[name] = nc.dram_tensor(name, shape, dt, kind="ExternalInput")
        return dram[name]

    xT_d = din("xT", [C, T], F32)
    xTmine_d = din("xTmine", [C, MYT], F32)
    wq_d = din("wq", [C, 512], BF16)
    wk_d = din("wk", [C, 512], BF16)
    wv_d = din("wv", [C, 512], BF16)
    wproj_d = din("wproj", [C, C], BF16)
    wff1_d = din("wff1", [C, FF], BF16)
    wff2_d = din("wff2", [FF, C], BF16)
    bqkv_d = din("bqkv", [P, 12], F32)
    bproj_d = din("bproj", [P, 8], F32)
    bff1_d = din("bff1", [P, 32], F32)
    bff2_d = din("bff2", [P, 8], F32)
    trimask_d = din("trimask", [P, P], F32)
    onesb_d = din("onesb", [P, 1], BF16)
    yT_d = nc.dram_tensor("yT", [C, MYT], F32, kind="ExternalOutput")
    dbg = {}
    if DEBUG_TAPS:
        dbg["xn"] = nc.dram_tensor("dbg_xn", [P, CK * T], BF16, kind="ExternalOutput")
        dbg["qT"] = nc.dram_tensor("dbg_qT", [P, 4 * T], BF16, kind="ExternalOutput")
        dbg["kT"] = nc.dram_tensor("dbg_kT", [P, 4 * T], BF16, kind="ExternalOutput")
        dbg["v"] = nc.dram_tensor("dbg_v", [P, 16 * 512], BF16, kind="ExternalOutput")
        dbg["attn"] = nc.dram_tensor("dbg_attn", [P, 4 * T], BF16, kind="ExternalOutput")
        dbg["x1"] = nc.dram_tensor("dbg_x1", [P, CK * MYT], F32, kind="ExternalOutput")
        dbg["s2"] = nc.dram_tensor("dbg_s2", [P, 4 * 1024], F32, kind="ExternalOutput")
        dbg["p2"] = nc.dram_tensor("dbg_p2", [P, 4 * 1024], BF16, kind="ExternalOutput")
        dbg["rdb"] = nc.dram_tensor("dbg_rdb", [P, 512], F32, kind="ExternalOutput")
        dbg["dd"] = nc.dram_tensor("dbg_dd", [P, 2048], F32, kind="ExternalOutput")

    with tile.TileContext(nc) as tc:
        _emit(nc, tc, dram, yT_d, dbg)
    nc.compile()
    return nc


def _emit(nc, tc, dram, yT_d, dbg):
    sync, vec, act, gps, pe = nc.sync, nc.vector, nc.scalar, nc.gpsimd, nc.tensor

    pid = nc.sync.partition_id()
    off_mine = (pid % 2) * MYT          # my token half, in columns
    off_send = MYT - (pid % 2) * MYT    # partner's token half
    slot_partner = 1 - (pid % 2)        # partner's rank slot in the AllGather

    def layernorm_stats(xsrc, ntok, statp, smp, prefix, on_ready=None):
        """xsrc: callable (ck, tcx) -> bf16 [P, 512] tile AP for that chunk.
        Returns per-512-token-chunk (mean_b, rstd_b) f32 [P, 512] broadcast tiles,
        emitted tcx-granular so downstream consumers of early chunks can start
        while later chunks' stats are still being accumulated."""
        ntc = ntok // 512
        out = []
        for tcx in range(ntc):
            stat = statp.tile([64, 512], F32, tag="stat", name=f"{prefix}st{tcx}")
            for ck in range(CK):
                xbt = xsrc(ck, tcx)
                xsq = smp.tile([P, 512], BF16, tag="xsq", name=f"{prefix}xsq{tcx}_{ck}")
                act.square(xsq[:], xbt[:])
                pe.matmul(stat[0:1, :], onesb[:], xbt[:],
                          start=(ck == 0), stop=False)
                pe.matmul(stat[32:33, :], onesb[:], xsq[:],
                          start=(ck == 0), stop=(ck == CK - 1),
                          skip_group_check=True)
            mean = smp.tile([1, 512], F32, tag="sm", name=f"{prefix}mean{tcx}")
            act.mul(mean[:], stat[0:1, :], 1.0 / C)
            msq = smp.tile([1, 512], F32, tag="sm", name=f"{prefix}msq{tcx}")
            act.mul(msq[:], stat[32:33, :], 1.0 / C)
            var = smp.tile([1, 512], F32, tag="sm", name=f"{prefix}var{tcx}")
            act.square(var[:], mean[:])
            vec.tensor_tensor(var[:], msq[:], var[:], OP.subtract)
            std = smp.tile([1, 512], F32, tag="sm", name=f"{prefix}std{tcx}")
            act.activation(std[:], var[:], AF.Sqrt, bias=_eps[0:1, :])
            rstd = smp.tile([1, 512], F32, tag="sm", name=f"{prefix}rstd{tcx}")
            vec.reciprocal_approx_fast(rstd[:], std[:])
            mb = smp.tile([P, 512], F32, tag=f"mb{tcx}", name=f"{prefix}mb{tcx}", bufs=1)
            rb = smp.tile([P, 512], F32, tag=f"rb{tcx}", name=f"{prefix}rb{tcx}", bufs=1)
            gps.partition_broadcast(mb[:], mean[:])
            gps.partition_broadcast(rb[:], rstd[:])
            out.append((mb, rb))
            if on_ready is not None:
                on_ready(tcx, mb, rb)
        return out

    with (
        tc.tile_pool(name="const", bufs=1) as constp,
        tc.tile_pool(name="mid", bufs=1) as midp,
        tc.tile_pool(name="dram", bufs=1, space="DRAM") as dramp,
    ):
        trimask = constp.tile([P, P], F32)
        sync.dma_start(trimask[:], dram["trimask"].ap())
        onesb = constp.tile([P, 1], BF16)
        sync.dma_start(onesb[:], dram["onesb"].ap())
        bqkv = constp.tile([P, 12], F32)
        sync.dma_start(bqkv[:], dram["bqkv"].ap())
        bproj = constp.tile([P, 8], F32)
        sync.dma_start(bproj[:], dram["bproj"].ap())
        bff1 = constp.tile([P, 32], F32)
        sync.dma_start(bff1[:], dram["bff1"].ap())
        bff2 = constp.tile([P, 8], F32)
        sync.dma_start(bff2[:], dram["bff2"].ap())
        epst = constp.tile([P, 1], F32)
        vec.memset(epst[:], LN_EPS)
        _eps = epst

        def _ones(_ck):
            return onesb[:]

        # tensors that must outlive the attention scope
        amine3 = midp.tile([P, 4, MYT], BF16, tag="amine")
        apart3 = midp.tile([P, 4, MYT], BF16, tag="apart")
        xn23 = midp.tile([P, CK, MYT], BF16, tag="xn2")

        with tc.tile_pool(name="attnspan", bufs=1) as attnsp:
            attn3 = attnsp.tile([P, 4, T], BF16)

            with tc.tile_pool(name="qkvxn", bufs=1) as qxp:
                qT3 = qxp.tile([P, 4, T], BF16, tag="qT3")
                kT3 = qxp.tile([P, 4, T], BF16, tag="kT3")
                v3 = qxp.tile([P, 16, 512], BF16, tag="v3")

                with (
                    tc.tile_pool(name="span_xn", bufs=1) as xnp,
                    tc.tile_pool(name="ln1_rot", bufs=2) as lp,
                    tc.tile_pool(name="ln1_small", bufs=5) as smp,
                    tc.tile_pool(name="ln1_stats", bufs=2, space="PSUM") as statp,
                    tc.tile_pool(name="wqkv", bufs=1) as wp,
                    tc.tile_pool(name="qkv_ps", bufs=4, space="PSUM") as qp,
                ):
                    xn3 = xnp.tile([P, CK, T], BF16, tag="xn3")
                    wq = wp.tile([P, CK, 512], BF16, tag="wq")
                    sync.dma_start(wq[:], dram["wq"].ap().rearrange("(ck p) n -> p ck n", p=P))
                    wk = wp.tile([P, CK, 512], BF16, tag="wk")
                    sync.dma_start(wk[:], dram["wk"].ap().rearrange("(ck p) n -> p ck n", p=P))
                    wv = wp.tile([P, CK, 512], BF16, tag="wv")
                    sync.dma_start(wv[:], dram["wv"].ap().rearrange("(ck p) n -> p ck n", p=P))

                    def xsrc1(ck, tcx):
                        xt = lp.tile([P, 512], F32, tag="xT", name=f"xt{tcx}_{ck}")
                        sync.dma_start(xt[:], dram["xT"].ap()[ts(ck, P), ts(tcx, 512)])
                        xbt = lp.tile([P, 512], BF16, tag="xb", name=f"xb{tcx}_{ck}")
                        vec.tensor_copy(xbt[:], xt[:])
                        return xbt

                    def ln1_ready(tcx, mb, rb):
                        # normalize this 512-token chunk, then its QKV slice
                        for ck in range(CK):
                            xt = lp.tile([P, 512], F32, tag="xT", name=f"xtn{tcx}_{ck}")
                            sync.dma_start(xt[:], dram["xT"].ap()[ts(ck, P), ts(tcx, 512)])
                            tmp = lp.tile([P, 512], BF16, tag="lntmp", name=f"lt{ck}_{tcx}")
                            vec.tensor_tensor(tmp[:], xt[:], mb[:], OP.subtract)
                            vec.tensor_tensor(xn3[:, ck, ts(tcx, 512)], tmp[:], rb[:], OP.mult)
                        for ti, (wt, out3) in enumerate(((wq, qT3), (wk, kT3))):
                            for m in range(4):
                                ps = qp.tile([P, 512], F32, tag="qkps",
                                             name=f"qk{ti}_{m}_{tcx}")
                                for ck in range(CK):
                                    pe.matmul(ps[:], wt[:, ck, ts(m, P)],
                                              xn3[:, ck, ts(tcx, 512)],
                                              start=(ck == 0), stop=(ck == CK - 1))
                                vec.tensor_scalar_add(out3[:, m, ts(tcx, 512)], ps[:],
                                                      bqkv[:, ti * 4 + m : ti * 4 + m + 1])
                        for kt in range(4 * tcx, 4 * tcx + 4):
                            ps = qp.tile([P, 512], F32, tag="qkps", name=f"v{kt}")
                            for ck in range(CK):
                                pe.matmul(ps[:], xn3[:, ck, ts(kt, P)], wv[:, ck, :],
                                          start=(ck == 0), stop=(ck == CK - 1))
                            vec.tensor_copy(v3[:, kt, :], ps[:])

                    layernorm_stats(xsrc1, T, statp, smp, "l1", on_ready=ln1_ready)

                # ----------------------------------------------------------
                # Phase 3: causal attention per (head-pair hp, q-tile qt)                # ----------------------------------------------------------
                # Phase 3: causal attention per (head-pair hp, q-tile qt)
                # ----------------------------------------------------------
                cins = [dramp.tile([P, MYT], BF16, name=f"cin{h_}") for h_ in range(4)]
                couts = [dramp.tile([2, P, MYT], BF16, name=f"cout{h_}") for h_ in range(4)]
                w1cm = tc.tile_pool(name="w1", bufs=1, side="right")
                w1p = w1cm.__enter__()
                w1 = w1p.tile([P, CK, FF], BF16)
                sync.dma_start(w1[:], dram["wff1"].ap().rearrange("(ck p) n -> p ck n", p=P))
                with (
                    tc.tile_pool(name="p2", bufs=6) as p2p,
                    tc.tile_pool(name="rd", bufs=2) as rdp,
                    tc.tile_pool(name="ps_s", bufs=2, space="PSUM") as psS,
                    tc.tile_pool(name="ps_attn", bufs=2, space="PSUM") as psA,
                    tc.tile_pool(name="ps_d", bufs=1, space="PSUM") as psD,
                ):
                    for hp in range(4):
                        for qt in range(4):
                            t0 = qt * 512
                            kdiag = 4 * qt
                            chunks = ([kdiag] + list(range(kdiag))
                                      + [kdiag + 1, kdiag + 2, kdiag + 3])
                            attn2 = psA.tile([P, 512], F32, tag="attn2", bufs=1,
                                              name=f"at{hp}_{qt}")
                            attn2b = psA.tile([P, 512], F32, tag="attn2b", bufs=1,
                                              name=f"atb{hp}_{qt}")
                            d2 = psD.tile([64, 512], F32, tag="d2", bufs=2,
                                          name=f"d{hp}_{qt}")
                            s_tiles = {}

                            def qk(k):
                                off = 0 if k <= kdiag else P * (k - kdiag)
                                s2 = psS.tile([P, 1024], F32, tag="s2",
                                              name=f"s{hp}_{qt}_{k}")
                                pe.matmul(s2[:, off:512],
                                          kT3[0:64, hp, ts(k, P)],
                                          qT3[0:64, hp, t0 + off : t0 + 512],
                                          start=True, stop=True, tile_position=(0, 0))
                                pe.matmul(s2[:, 512 + off : 1024],
                                          kT3[64:128, hp, ts(k, P)],
                                          qT3[64:128, hp, t0 + off : t0 + 512],
                                          start=True, stop=True, tile_position=(64, 0))
                                s_tiles[k] = s2

                            qk(chunks[0])
                            if len(chunks) > 1:
                                qk(chunks[1])
                            for i, k in enumerate(chunks):
                                off = 0 if k <= kdiag else P * (k - kdiag)
                                first = i == 0
                                last = i == len(chunks) - 1
                                s2 = s_tiles.pop(k)
                                if k >= kdiag:  # diagonal chunk: triangular mask
                                    vec.tensor_tensor(s2[:, off : off + P],
                                                      s2[:, off : off + P],
                                                      trimask[:], OP.add)
                                    vec.tensor_tensor(s2[:, 512 + off : 512 + off + P],
                                                      s2[:, 512 + off : 512 + off + P],
                                                      trimask[:], OP.add)
                                p2 = p2p.tile([P, 1024], BF16, tag="p2",
                                              name=f"p{hp}_{qt}_{k}")
                                if dbg and hp == 0 and qt == 0:
                                    s2c = p2p.tile([P, 1024], F32, tag="s2c",
                                                   name=f"s2c{k}")
                                    vec.tensor_copy(s2c[:], s2[:])
                                    sync.dma_start(
                                        dbg["s2"].ap().rearrange("p (k n) -> p k n", k=4)[:, k, :],
                                        s2c[:])
                                if off == 0:
                                    act.activation(p2[:, 0:1024], s2[:, 0:1024], AF.Exp)
                                else:
                                    act.activation(p2[:, off:512], s2[:, off:512], AF.Exp)
                                    act.activation(p2[:, 512 + off : 1024],
                                                   s2[:, 512 + off : 1024], AF.Exp)
                                if dbg and hp == 0 and qt == 0:
                                    sync.dma_start(
                                        dbg["p2"].ap().rearrange("p (k n) -> p k n", k=4)[:, k, :],
                                        p2[:])
                                if i + 2 < len(chunks):
                                    qk(chunks[i + 2])
                                vw = v3[:, k, hp * P : hp * P + P]
                                pe.matmul(attn2[:, off:512], vw, p2[:, off:512],
                                          start=first, stop=last)
                                pe.matmul(attn2b[:, off:512], vw,
                                          p2[:, 512 + off : 1024],
                                          start=first, stop=last)
                                pe.matmul(d2[0:1, off:512], onesb[:], p2[:, off:512],
                                          start=first, stop=last,
                                          tile_position=(0, 0))
                                pe.matmul(d2[32:33, off:512], onesb[:],
                                          p2[:, 512 + off : 1024],
                                          start=first, stop=last,
                                          tile_position=(0, 32), skip_group_check=True)

                            dsA = rdp.tile([1, 512], F32, tag="dsA", name=f"ca{hp}_{qt}")
                            dsB = rdp.tile([1, 512], F32, tag="dsB", name=f"cb{hp}_{qt}")
                            vec.tensor_copy(dsA[0:1, :], d2[0:1, :])
                            vec.tensor_copy(dsB[0:1, :], d2[32:33, :])
                            rdA = rdp.tile([1, 512], F32, tag="rdA", name=f"ra{hp}_{qt}")
                            rdB = rdp.tile([1, 512], F32, tag="rdB", name=f"rB{hp}_{qt}")
                            vec.reciprocal_approx_fast(rdA[0:1, :], dsA[0:1, :])
                            vec.reciprocal_approx_fast(rdB[0:1, :], dsB[0:1, :])
                            rdbA = rdp.tile([P, 512], F32, tag="rdbA", name=f"rba{hp}_{qt}")
                            rdbB = rdp.tile([P, 512], F32, tag="rdbB", name=f"rbb{hp}_{qt}")
                            gps.partition_broadcast(rdbA[:, :], rdA[0:1, :])
                            gps.partition_broadcast(rdbB[:, :], rdB[0:1, :])
                            if dbg and hp == 0 and qt == 0:
                                sync.dma_start(dbg["rdb"].ap()[0:64, :], rdbA[0:64, :])
                                sync.dma_start(dbg["rdb"].ap()[64:128, :], rdbB[64:128, :])
                                ddc = rdp.tile([1, 2048], F32, tag="ddc", name="ddc0")
                                vec.tensor_copy(ddc[0:1, 0:512], dsA[0:1, :])
                                vec.tensor_copy(ddc[0:1, 512:1024], dsB[0:1, :])
                                vec.tensor_copy(ddc[0:1, 1024:1536], rdA[0:1, :])
                                vec.tensor_copy(ddc[0:1, 1536:2048], rdB[0:1, :])
                                sync.dma_start(dbg["dd"].ap()[0:1, :], ddc[0:1, :])
                            vec.tensor_tensor(attn3[0:64, hp, ts(qt, 512)],
                                              attn2[0:64, :], rdbA[0:64, :], OP.mult)
                            vec.tensor_tensor(attn3[64:128, hp, ts(qt, 512)],
                                              attn2b[64:128, :], rdbB[64:128, :], OP.mult)

                        # per-hp exchange as soon as this head-pair finishes
                        sync.dma_start(cins[hp][:, :], attn3[:, hp, ds(off_send, MYT)])
                        sync.dma_start(amine3[:, hp, :], attn3[:, hp, ds(off_mine, MYT)])
                        if SIM_SINGLE:
                            sync.dma_start(couts[hp][0], cins[hp][:])
                            sync.dma_start(couts[hp][1], cins[hp][:])
                        else:
                            gps.collective_compute(
                                "AllGather",
                                OP.bypass,
                                ins=[cins[hp][:].opt()],
                                outs=[couts[hp][:].opt()],
                                replica_groups=[[0, 1], [2, 3], [4, 5], [6, 7]],
                            )

            if dbg:
                sync.dma_start(dbg["attn"].ap().rearrange("p (m t) -> p m t", m=4), attn3[:])

            # --------------------------------------------------------------
            # Phase 4 tail: read back per-hp AllGather results
            # --------------------------------------------------------------
            for hp in range(4):
                sync.dma_start(apart3[:, hp, :],
                               couts[hp][ds(slot_partner, 1), :, :])

        # ------------------------------------------------------------------
        # Phase 5-7: proj + residual, LN2, FFN  (my MYT tokens)
        # ------------------------------------------------------------------
        with tc.tile_pool(name="x1span", bufs=1) as x1p:
            x1T3 = x1p.tile([P, CK, MYT], F32)

            with (
                tc.tile_pool(name="wproj", bufs=1) as wpp,
                tc.tile_pool(name="xmine", bufs=1) as xmp,
                tc.tile_pool(name="proj_ps", bufs=4, space="PSUM") as pp,
            ):
                wproj = wpp.tile([P, CK, C], BF16)
                for r_ in range(CK):
                    sync.dma_start(wproj[:, r_, :], dram["wproj"].ap()[ts(r_, P), :])
                xmine = xmp.tile([P, CK, MYT], F32)
                for r_ in range(CK):
                    sync.dma_start(xmine[:, r_, :], dram["xTmine"].ap()[ts(r_, P), :])
                for tc2 in range(2):
                    for m in range(CK):
                        ps = pp.tile([P, 512], F32, tag="projps", name=f"pj{m}_{tc2}")
                        for r in range(4):
                            pe.matmul(ps[:], wproj[:, r, ts(m, P)],
                                      amine3[:, r, ts(tc2, 512)],
                                      start=(r == 0), stop=False)
                        for r in range(4):
                            pe.matmul(ps[:], wproj[:, r + 4, ts(m, P)],
                                      apart3[:, r, ts(tc2, 512)],
                                      start=False, stop=(r == 3))
                        vec.scalar_tensor_tensor(
                            x1T3[:, m, ts(tc2, 512)], ps[:], bproj[:, m : m + 1],
                            xmine[:, m, ts(tc2, 512)], op0=OP.add, op1=OP.add)

            if dbg:
                sync.dma_start(dbg["x1"].ap().rearrange("p (ck t) -> p ck t", ck=CK), x1T3[:])

            # LN2
            with (
                tc.tile_pool(name="ln2_rot", bufs=2) as lp2,
                tc.tile_pool(name="ln2_xb", bufs=1) as xb2p,
                tc.tile_pool(name="ln2_small", bufs=5) as smp2,
                tc.tile_pool(name="ln2_stats", bufs=2, space="PSUM") as statp2,
            ):
                xb23 = xb2p.tile([P, CK, MYT], BF16)

                def xsrc2(ck, tcx):
                    vec.tensor_copy(xb23[:, ck, ts(tcx, 512)], x1T3[:, ck, ts(tcx, 512)])
                    return xb23[:, ck, ts(tcx, 512)]

                def ln2_ready(tcx, mb, rb):
                    for ck in range(CK):
                        tmp = lp2.tile([P, 512], BF16, tag="lntmp2", name=f"l2t{ck}_{tcx}")
                        vec.tensor_tensor(tmp[:], x1T3[:, ck, ts(tcx, 512)], mb[:],
                                          OP.subtract)
                        vec.tensor_tensor(xn23[:, ck, ts(tcx, 512)], tmp[:], rb[:],
                                          OP.mult)
                layernorm_stats(xsrc2, MYT, statp2, smp2, "l2", on_ready=ln2_ready)

            # FFN
            with (
                tc.tile_pool(name="w2rot", bufs=4) as w2p,
                tc.tile_pool(name="h", bufs=1) as hp_,
                tc.tile_pool(name="yrot", bufs=3) as yp,
                tc.tile_pool(name="ffn_ps", bufs=2, space="PSUM") as fp,
                tc.tile_pool(name="ffn2_ps", bufs=1, space="PSUM") as fp2,
            ):
                for tc2 in range(2):
                    h3 = hp_.tile([P, 32, 512], BF16, tag="h3", name=f"h{tc2}")
                    for fm in range(32):
                        ps = fp.tile([P, 512], F32, tag="ffps", name=f"f{tc2}_{fm}")
                        for ck in range(CK):
                            pe.matmul(ps[:], w1[:, ck, ts(fm, P)],
                                      xn23[:, ck, ts(tc2, 512)],
                                      start=(ck == 0), stop=(ck == CK - 1))
                        act.activation(h3[:, fm, :], ps[:], AF.Relu,
                                       bias=bff1[:, fm : fm + 1])
                    for g in range(2):
                        ps_y = [fp2.tile([P, 512], F32, tag=f"yps{j}", bufs=1,
                                         name=f"y{tc2}_{g}_{j}") for j in range(4)]
                        for fk in range(32):
                            w2t = w2p.tile([P, C], BF16, tag="w2t", name=f"w2_{tc2}_{g}_{fk}")
                            sync.dma_start(w2t[:], dram["wff2"].ap()[ts(fk, P), :])
                            for j in range(4):
                                m2 = g * 4 + j
                                pe.matmul(ps_y[j][:], w2t[:, ts(m2, P)], h3[:, fk, :],
                                          start=(fk == 0), stop=(fk == 31))
                        for j in range(4):
                            m2 = g * 4 + j
                            yt = yp.tile([P, 512], F32, tag="yt", name=f"yo{tc2}_{g}_{j}")
                            vec.scalar_tensor_tensor(
                                yt[:], ps_y[j][:], bff2[:, m2 : m2 + 1],
                                x1T3[:, m2, ts(tc2, 512)], op0=OP.add, op1=OP.add)
                            sync.dma_start(yT_d.ap()[ts(m2, P), ts(tc2, 512)], yt[:])
            w1cm.__exit__(None, None, None)


# ----------------------------------------------------------------------------
# host-side input prep
# ----------------------------------------------------------------------------

def _bf16(a):
    return np.asarray(a, dtype=ml_dtypes.bfloat16)


def _prep_inputs(inputs):
    x = np.asarray(inputs["x"], np.float32)
    wq = np.asarray(inputs["wq"], np.float32)
    wk = np.asarray(inputs["wk"], np.float32)
    wv = np.asarray(inputs["wv"], np.float32)
    w_proj = np.asarray(inputs["w_proj"], np.float32)
    b_proj = np.asarray(inputs["b_proj"], np.float32)
    w_ff1 = np.asarray(inputs["w_ff1"], np.float32)
    b_ff1 = np.asarray(inputs["b_ff1"], np.float32)
    w_ff2 = np.asarray(inputs["w_ff2"], np.float32)
    b_ff2 = np.asarray(inputs["b_ff2"], np.float32)
    s1 = np.asarray(inputs["ln1_scale"], np.float32)
    b1 = np.asarray(inputs["ln1_bias"], np.float32)
    s2 = np.asarray(inputs["ln2_scale"], np.float32)
    b2 = np.asarray(inputs["ln2_bias"], np.float32)

    assert np.all(np.asarray(inputs["wv"]).shape == (H, C, D))
    scale = C ** -0.5

    # [H, C, D] -> [C, H*D], LN1 scale folded into rows; C^-0.5 folded into wq
    def flat(w, extra=1.0):
        return np.ascontiguousarray(w.transpose(1, 0, 2).reshape(C, H * D)) * extra

    wq_f = flat(wq, scale)        # [C, 1024]
    wk_f = flat(wk)
    wv_f = flat(wv)
    qb_full = b1 @ wq_f           # [1024] biases from ln1_bias (zero in practice)
    kb_full = b1 @ wk_f
    vb_full = b1 @ wv_f
    assert np.abs(vb_full).max() == 0.0, "nonzero ln1_bias @ wv not supported"
    wq_s = wq_f * s1[:, None]
    wk_s = wk_f * s1[:, None]
    wv_s = wv_f * s1[:, None]

    wff1_s = w_ff1 * s2[:, None]
    bff1_full = b2 @ w_ff1 + b_ff1    # [4096]

    trimask = np.where(np.arange(P)[:, None] <= np.arange(P)[None, :], 0.0, NEG
                       ).astype(np.float32)
    onesb = _bf16(np.ones((P, 1), np.float32))

    in_maps = []
    for c in range(NCORES):
        b, hg = c // 2, c % 2
        hsl = slice(hg * 512, hg * 512 + 512)
        perm = np.concatenate([np.arange(hg * 512, hg * 512 + 512),
                               np.arange((1 - hg) * 512, (1 - hg) * 512 + 512)])
        m = {
            "xT": np.ascontiguousarray(x[b].T),
            "xTmine": np.ascontiguousarray(x[b, hg * MYT:(hg + 1) * MYT].T),
            "wq": _bf16(wq_s[:, hsl]),
            "wk": _bf16(wk_s[:, hsl]),
            "wv": _bf16(wv_s[:, hsl]),
            "wproj": _bf16(w_proj[perm, :]),
            "wff1": _bf16(wff1_s),
            "wff2": _bf16(w_ff2),
            "bqkv": np.ascontiguousarray(np.concatenate(
                [qb_full[hsl].reshape(4, P).T,
                 kb_full[hsl].reshape(4, P).T,
                 np.zeros((P, 4), np.float32)], axis=1)),
            "bproj": np.ascontiguousarray(b_proj.reshape(8, P).T),
            "bff1": np.ascontiguousarray(bff1_full.reshape(32, P).T),
            "bff2": np.ascontiguousarray(b_ff2.reshape(8, P).T),
            "trimask": trimask,
            "onesb": onesb,
        }
        in_maps.append(m)
    return in_maps


# ----------------------------------------------------------------------------
# cached PJRT runner (one jit, reused across calls)
# ----------------------------------------------------------------------------

def _get_runner():
    with _lock:
        if "runner" in _cache:
            return _cache["runner"]

        import jax
        import jax.numpy as jnp
        from jax.sharding import Mesh, PartitionSpec
        from jax.experimental.shard_map import shard_map
        from concourse import bass2jax

        nc = _build_nc()
        bass2jax.install_neuronx_cc_hook()

        partition_name = nc.partition_id_tensor.name if nc.partition_id_tensor else None
        in_names, out_names, out_avals, zero_outs = [], [], [], []
        for alloc in nc.m.functions[0].allocations:
            if not isinstance(alloc, mybir.MemoryLocationSet):
                continue
            name = alloc.memorylocations[0].name
            if alloc.kind == "ExternalInput":
                if name != partition_name:
                    in_names.append(name)
            elif alloc.kind == "ExternalOutput":
                shape = tuple(alloc.tensor_shape)
                dtype = mybir.dt.np(alloc.dtype)
                out_names.append(name)
                out_avals.append(jax.core.ShapedArray(shape, dtype))
                zero_outs.append(np.zeros(shape, dtype))
        n_params = len(in_names)
        all_in_names = list(in_names) + list(out_names)
        if partition_name is not None:
            all_in_names.append(partition_name)
        donate = tuple(range(n_params, n_params + len(out_names)))

        def _body(*args):
            operands = list(args)
            if partition_name is not None:
                operands.append(bass2jax.partition_id_tensor())
            outs = bass2jax._bass_exec_p.bind(
                *operands,
                out_avals=tuple(out_avals),
                in_names=tuple(all_in_names),
                out_names=tuple(out_names),
                lowering_input_output_aliases=(),
                sim_require_finite=True,
                sim_require_nnan=True,
                nc=nc,
            )
            return tuple(outs)

        devices = jax.devices()[:NCORES]
        mesh = Mesh(np.asarray(devices), ("core",))
        in_specs = (PartitionSpec("core"),) * (n_params + len(out_names))
        out_specs = (PartitionSpec("core"),) * len(out_names)
        sharded = jax.jit(
            shard_map(_body, mesh=mesh, in_specs=in_specs, out_specs=out_specs,
                      check_rep=False),
            donate_argnums=donate, keep_unused=True,
        )

        from jax.sharding import NamedSharding

        def run(in_maps, timing_iters=0):
            per_core = [[np.asarray(m[k]) for k in in_names] for m in in_maps]
            concat_in = [np.concatenate([per_core[c][i] for c in range(NCORES)], axis=0)
                         for i in range(n_params)]
            shard = NamedSharding(mesh, PartitionSpec("core"))
            dev_in = [jax.device_put(a, shard) for a in concat_in]
            jax.block_until_ready(dev_in)

            def dev_zeros():
                zz = [jax.device_put(
                    np.zeros((NCORES * z.shape[0], *z.shape[1:]), z.dtype), shard)
                    for z in zero_outs]
                jax.block_until_ready(zz)
                return zz

            out = sharded(*dev_in, *dev_zeros())
            jax.block_until_ready(out)
            times = []
            if timing_iters:
                import time
                for _ in range(timing_iters):
                    zz = dev_zeros()
                    t0 = time.perf_counter()
                    o2 = sharded(*dev_in, *zz)
                    jax.block_until_ready(o2)
                    times.append(time.perf_counter() - t0)
                    out = o2
            res = [
                {name: np.asarray(out[i]).reshape(NCORES, *out_avals[i].shape)[c]
                 for i, name in enumerate(out_names)}
                for c in range(NCORES)
            ]
            return res, times

        _cache["runner"] = run
        return run


def kernel(**inputs) -> np.ndarray:
    run = _get_runner()
    in_maps = _prep_inputs(inputs)
    res, _ = run(in_maps)
    out = np.empty((B, T, C), np.float32)
    for c in range(NCORES):
        b, hg = c // 2, c % 2
        out[b, hg * MYT:(hg + 1) * MYT, :] = res[c]["yT"].T
    return out


def benchmark(inputs, iters=10):
    """Returns (output, list of per-iteration wall seconds)."""
    run = _get_runner()
    in_maps = _prep_inputs(inputs)
    res, times = run(in_maps, timing_iters=iters)
    out = np.empty((B, T, C), np.float32)
    for c in range(NCORES):
        b, hg = c // 2, c % 2
        out[b, hg * MYT:(hg + 1) * MYT, :] = res[c]["yT"].T
    return out, times
